# revision 14
# baseline (speedup 1.0000x reference)
# Trainium2 Bass kernel for nn_CosSimRouter_pad.
#
# Strategy (8 NeuronCores, SPMD, no collectives):
#   Device program 1 ("scores"): cos = normalize(vision) @ normalize(text).T
#     sharded over the text dim (1024 text rows per core, two 512-col halves).
#     Inputs are cast to fp8e4m3 (scaled by 64 so entries sit in the normal
#     range) and the matmul runs in DoubleRow perf mode: 256-deep contraction
#     tiles at 2 MACs/cell/cycle, so the PE does half the instructions of the
#     bf16-rate path. Each core emits the top-2 argmax indices per (vision
#     token, half) -- the vector engine's MAX8 reduction gives the top-8 for
#     free -- batched into a single 40KB DMA. The host rescores those
#     candidate indices exactly in fp64, so fp8 matmul noise never reaches
#     the (discrete) selection: top-2 rescue reproduces the fp32 scores to
#     3e-8 on this input distribution, vs selection margins of ~1e-3.
#   Host: softmax/argsort/cumsum threshold selection, neighbor expansion,
#     unique, then the small [S,576] cos-sim + top-16 + softmax weights are
#     computed with jax on CPU using the exact op sequence of the original
#     module so the (discrete) selection matches it bit-for-bit. The weights
#     are scattered into a dense row-sparse matrix W [576, 576].
#   Device program 2 ("pool"): out = W @ vision_feature in bf16 with bf16
#     output (total ~4e-3 rel err vs the 2e-2 gate), sharded over output
#     columns (512 per core).
#
# Both programs issue a stream of tiny warm-up matmuls on zeroed SBUF while
# the first input DMAs are in flight, so the PE's HAM clock gate reaches
# 8/8 (2.4 GHz) before the real matmuls start instead of ~3.4us into them.

import os

os.environ.setdefault("MYCRO_LOCAL_CACHE", "1")

import numpy as np
import ml_dtypes

GAMMA = 0.5
TEMP = 0.05
TOP_K = 16
PAD = 1
GRID = 24
EPS = 1e-8

LV = 576          # vision tokens
LT = 8192         # text tokens
D = 4096          # embed dim
NCORES = 8
LT_SH = LT // NCORES          # 1024 text rows per core
KT2 = D // 256                # 16 DoubleRow contraction tiles (256 deep each)
NH = 2                        # 512-wide halves of the 1024-wide shard
M_TILES = (128, 128, 128, 128, 64)   # 576 = 4*128 + 64
NM = len(M_TILES)
KV = 5                        # ceil(576/128) contraction tiles for program 2
TOPC = 2                      # top-2 candidate indices per (token, half)
F8_SCALE = 64.0               # 2^6: lifts ~N(0,1/64^2) entries into e4m3 range
N_WARM_SCORES = 32            # tiny-matmul warmup while input DMAs fly; more
N_WARM_POOL = 32              # delays real MMs (they queue behind on the PE)


def _warmup(nc, tile_ctx_pools, psum_tag, n_warm, width=16):
    """Issue n_warm tiny matmuls on zeroed SBUF to warm the PE clock."""
    import concourse.mybir as mybir

    warm_pool, psum_pool = tile_ctx_pools
    f32 = mybir.dt.float32
    bf16 = mybir.dt.bfloat16
    wz = warm_pool.tile([128, width], bf16)
    nc.vector.memzero(wz)
    # shares the real psum tag so it joins that rotation instead of
    # claiming bufs x banks of its own
    wp = psum_pool.tile([128, width], f32, name="warm_ps", tag=psum_tag)
    for _ in range(n_warm):
        nc.tensor.matmul(wp[:width, :], lhsT=wz[:, :], rhs=wz[:, :],
                         start=True, stop=True)


def _build_scores_nc():
    """Per text shard: top-2 argmax of (vnT.T @ tnT) over each 512-col half.

    fp8e4m3 DoubleRow matmul; the host rescores the returned candidate
    indices exactly, so device numerics only need to keep the true argmax
    inside the top-2 -- a ~50x margin on this input."""
    import concourse.mybir as mybir
    import concourse.tile as tile
    from concourse import bacc

    nc = bacc.Bacc(
        "TRN2",
        target_bir_lowering=False,
        debug=False,
        enable_asserts=True,
        num_devices=NCORES,
    )
    DR = mybir.MatmulPerfMode.DoubleRow
    mmdt = mybir.dt.float8e4
    f32 = mybir.dt.float32
    u32 = mybir.dt.uint32
    # DoubleRow plane-separated layouts: global k = K*256 + o*128 + p.
    # k/o/free are adjacent dims, so each partition's chunk is one
    # contiguous DRAM run (1-4KB per descriptor).
    vnT = nc.dram_tensor("vnT", [128, KT2, 2, LV], mmdt, kind="ExternalInput").ap()
    tnT = nc.dram_tensor("tnT", [NH, 128, KT2, 2, 512], mmdt,
                         kind="ExternalInput").ap()
    amax = nc.dram_tensor("amax", [128, NH * NM * 8], u32,
                          kind="ExternalOutput").ap()

    # laddered chunk sizes (in 256-deep k-tiles): small first chunks so the
    # first matmul starts early; few chunks so DMA trigger/completion
    # overhead (and the end-of-program event-semaphore count) stays small
    CHUNKS = (2, 2, 4, 8)
    assert sum(CHUNKS) == KT2

    with tile.TileContext(nc) as tc:
        with (
            tc.tile_pool(name="vn", bufs=1) as vn_pool,
            tc.tile_pool(name="tn", bufs=1) as tn_pool,
            tc.tile_pool(name="red", bufs=1) as red_pool,
            tc.tile_pool(name="psum", bufs=8, space="PSUM") as psum_pool,
        ):
            _warmup(nc, (red_pool, psum_pool), "ps", N_WARM_SCORES)

            vn_sb = vn_pool.tile([128, KT2, 2, LV], mmdt)
            tn_sb = [tn_pool.tile([128, KT2, 2, 512], mmdt, name=f"tn_{n}")
                     for n in range(NH)]
            for n in range(NH):
                kc = 0
                for ch in CHUNKS:
                    if n == 0:
                        # vn chunk loads ride the scalar HWDGE queue
                        nc.scalar.dma_start(
                            vn_sb[:, kc : kc + ch], vnT[:, kc : kc + ch]
                        )
                    nc.sync.dma_start(
                        tn_sb[n][:, kc : kc + ch], tnT[n, :, kc : kc + ch]
                    )
                    kc += ch

            # all 10 top-8 index vectors land here; one DMA ships them
            mi_all = red_pool.tile([128, NH * NM * 8], u32)

            def reduce(n, m, pm, psum):
                row = red_pool.tile([128, 512], f32, name=f"row_{n}_{m}")
                nc.scalar.copy(row[:pm, :], psum[:pm, :])
                mx = red_pool.tile([128, 8], f32, name=f"mx_{n}_{m}")
                off = (n * NM + m) * 8
                nc.vector.max(out=mx[:pm, :], in_=row[:pm, :])
                nc.vector.max_index(
                    out=mi_all[:pm, off : off + 8],
                    in_max=mx[:pm, :],
                    in_values=row[:pm, :],
                )

            # half 0: k-outer so the matmuls chase the arriving DMA chunks;
            # its reduction overlaps the half-1 matmul stream
            psums0 = [
                psum_pool.tile([128, 512], f32, name=f"ps_0_{m}", tag="ps")
                for m in range(NM)
            ]
            for k in range(KT2):
                for m, pm in enumerate(M_TILES):
                    nc.tensor.matmul(
                        psums0[m][:pm, :],
                        lhsT=vn_sb[:, k, :, m * 128 : m * 128 + pm],
                        rhs=tn_sb[0][:, k, :, :],
                        start=(k == 0),
                        stop=(k == KT2 - 1),
                        perf_mode=DR,
                    )
            for m, pm in enumerate(M_TILES):
                reduce(0, m, pm, psums0[m])

            # half 1 runs from resident SBUF: m-outer so each m-tile's
            # reduction hides under the next m-tile's matmuls, leaving only
            # the (64-row) last tile's reduction on the tail
            for m, pm in enumerate(M_TILES):
                ps = psum_pool.tile([128, 512], f32, name=f"ps_1_{m}", tag="ps")
                for k in range(KT2):
                    nc.tensor.matmul(
                        ps[:pm, :],
                        lhsT=vn_sb[:, k, :, m * 128 : m * 128 + pm],
                        rhs=tn_sb[1][:, k, :, :],
                        start=(k == 0),
                        stop=(k == KT2 - 1),
                        perf_mode=DR,
                    )
                reduce(1, m, pm, ps)
            nc.sync.dma_start(amax, mi_all)

    nc.compile()
    return nc


def _build_pool_nc():
    """out[:, c*512:(c+1)*512] = (W @ vf) for this core's 512-column slice.

    Column sharding: each core gets the full (small) W but only a 512-wide
    slice of vf. bf16 in and out halves both DMA directions."""
    import concourse.mybir as mybir
    import concourse.tile as tile
    from concourse import bacc

    nc = bacc.Bacc(
        "TRN2",
        target_bir_lowering=False,
        debug=False,
        enable_asserts=True,
        num_devices=NCORES,
    )
    mmdt = mybir.dt.bfloat16
    f32 = mybir.dt.float32
    wT = nc.dram_tensor("wT", [KV, 128, LV], mmdt, kind="ExternalInput").ap()
    vf = nc.dram_tensor("vf", [KV, 128, 512], mmdt, kind="ExternalInput").ap()
    out = nc.dram_tensor("out", [LV, 512], mmdt, kind="ExternalOutput").ap()

    CHUNKS = (2, 3)
    assert sum(CHUNKS) == KV

    with tile.TileContext(nc) as tc:
        with (
            tc.tile_pool(name="w", bufs=1) as w_pool,
            tc.tile_pool(name="vfp", bufs=1) as vf_pool,
            tc.tile_pool(name="ob", bufs=5) as out_pool,
            tc.tile_pool(name="psum", bufs=5, space="PSUM") as psum_pool,
        ):
            _warmup(nc, (out_pool, psum_pool), "pps", N_WARM_POOL, width=128)

            w_sb = w_pool.tile([128, KV, LV], mmdt)
            vf_sb = vf_pool.tile([128, KV, 512], mmdt)
            kc = 0
            for ch in CHUNKS:
                nc.sync.dma_start(w_sb[:, kc : kc + ch, :], wT[kc : kc + ch])
                nc.scalar.dma_start(vf_sb[:, kc : kc + ch, :], vf[kc : kc + ch])
                kc += ch

            for m, pm in enumerate(M_TILES):
                ps = psum_pool.tile([128, 512], f32, name=f"pps{m}", tag="pps")
                for k in range(KV):
                    nc.tensor.matmul(
                        ps[:pm, :],
                        lhsT=w_sb[:, k, m * 128 : m * 128 + pm],
                        rhs=vf_sb[:, k, :],
                        start=(k == 0),
                        stop=(k == KV - 1),
                    )
                ot = out_pool.tile([128, 512], mmdt, name=f"pot{m}", tag="pot")
                nc.vector.tensor_copy(ot[:pm, :], ps[:pm, :])
                # alternate rings so the output transfers overlap
                eng = nc.sync if m % 2 == 0 else nc.scalar
                eng.dma_start(out[m * 128 : m * 128 + pm, :], ot[:pm, :])

    nc.compile()
    return nc


_cache: dict = {}


def _get_nc(which: str):
    if which not in _cache:
        _cache[which] = (_build_scores_nc() if which == "scores"
                         else _build_pool_nc())
    return _cache[which]


class _Runner:
    """Cached PJRT executor for one Bass program across the 8 cores.

    Mirrors bass2jax.run_bass_via_pjrt's multi-core branch, but builds the
    jitted shard_map once (that function re-traces and re-compiles on every
    call) and lets chosen inputs be replicated instead of concatenated.

    Call with a dict: sharded inputs as global arrays (axis 0 = n_cores *
    per-core axis 0), replicated inputs at their per-core shape. Returns
    {name: global ndarray} with outputs concatenated along axis 0.
    """

    def __init__(self, nc, replicated=()):
        import jax
        from jax.experimental.shard_map import shard_map
        from jax.sharding import Mesh, PartitionSpec

        import concourse.mybir as mybir
        from concourse import bass2jax

        bass2jax.install_neuronx_cc_hook()
        assert not nc.has_collectives and nc.dbg_addr is None
        self.nc = nc
        part_name = nc.partition_id_tensor.name if nc.partition_id_tensor else None
        in_names, out_names, out_avals = [], [], []
        for alloc in nc.m.functions[0].allocations:
            if not isinstance(alloc, mybir.MemoryLocationSet):
                continue
            name = alloc.memorylocations[0].name
            if alloc.kind == "ExternalInput":
                if name != part_name:
                    in_names.append(name)
            elif alloc.kind == "ExternalOutput":
                out_names.append(name)
                out_avals.append(
                    jax.core.ShapedArray(
                        tuple(alloc.tensor_shape), mybir.dt.np(alloc.dtype)
                    )
                )
        self.in_names, self.out_names, self.out_avals = in_names, out_names, out_avals
        self.replicated = set(replicated)
        n_params = len(in_names)
        donate = tuple(range(n_params, n_params + len(out_names)))

        bind_names = in_names + out_names + ([part_name] if part_name else [])

        def _body(*args):
            operands = list(args)
            if part_name is not None:
                operands.append(bass2jax.partition_id_tensor())
            outs = bass2jax._bass_exec_p.bind(
                *operands,
                out_avals=tuple(out_avals),
                in_names=tuple(bind_names),
                out_names=tuple(out_names),
                lowering_input_output_aliases=(),
                sim_require_finite=True,
                sim_require_nnan=True,
                nc=nc,
            )
            return tuple(outs)

        devices = jax.devices()[:NCORES]
        mesh = Mesh(np.asarray(devices), ("core",))
        in_specs = tuple(
            PartitionSpec() if n in self.replicated else PartitionSpec("core")
            for n in in_names
        ) + (PartitionSpec("core"),) * len(out_names)
        out_specs = (PartitionSpec("core"),) * len(out_names)
        self._fn = jax.jit(
            shard_map(
                _body,
                mesh=mesh,
                in_specs=in_specs,
                out_specs=out_specs,
                check_rep=False,
            ),
            donate_argnums=donate,
            keep_unused=True,
        )

    def __call__(self, inputs: dict):
        args = [np.ascontiguousarray(inputs[n]) for n in self.in_names]
        zeros = [
            np.zeros((NCORES * a.shape[0], *a.shape[1:]), a.dtype)
            for a in self.out_avals
        ]
        outs = self._fn(*args, *zeros)
        return {n: np.asarray(o) for n, o in zip(self.out_names, outs)}


_runners: dict = {}


def _get_runner(which: str) -> _Runner:
    if which not in _runners:
        repl = {"scores": ("vnT",), "pool": ("wT",)}[which]
        _runners[which] = _Runner(_get_nc(which), replicated=repl)
    return _runners[which]


def _neighbor_unique(sel: np.ndarray) -> np.ndarray:
    offs = np.array(
        [
            [i, j]
            for i in range(-PAD, PAD + 1)
            for j in range(-PAD, PAD + 1)
            if not (i == 0 and j == 0)
        ],
        dtype=np.int64,
    )
    coords = np.stack([sel // GRID, sel % GRID], axis=1)
    padded = np.clip(coords[:, None, :] + offs[None, :, :], 0, GRID - 1)
    return np.unique(padded[..., 0] * GRID + padded[..., 1])


def kernel(vision_feature, text_embed, attention_mask):
    import jax
    import jax.numpy as jnp

    cpu = jax.devices("cpu")[0]

    vision_feature = np.asarray(vision_feature, dtype=np.float32)
    text_embed = np.asarray(text_embed, dtype=np.float32)
    mask_np = np.asarray(attention_mask)

    with jax.default_device(cpu):
        # normalize exactly as the reference does (jnp on CPU)
        vfj = jnp.asarray(vision_feature)
        tej = jnp.asarray(text_embed)
        vn = np.asarray(
            vfj / jnp.maximum(jnp.linalg.norm(vfj, axis=-1, keepdims=True), EPS)
        )
        tn = np.asarray(
            tej / jnp.maximum(jnp.linalg.norm(tej, axis=-1, keepdims=True), EPS)
        )

    # fold the attention mask into the text rows: where(mask, cos, 0) ==
    # cos * mask elementwise, and max over the text dim commutes with the
    # per-vision positive scale, so pre-scaling text rows by mask is exact.
    tns = tn * mask_np.astype(np.float32)[:, None]

    # ---- device program 1: sharded fp8 cos-sim + per-half top-2 argmax ----
    f8 = ml_dtypes.float8_e4m3
    # vnT[p, K, o, m] = vn[m, K*256 + o*128 + p] (scaled into e4m3 range)
    vnT = np.ascontiguousarray(
        (vn.T * F8_SCALE).reshape(KT2, 2, 128, LV).transpose(2, 0, 1, 3)
    ).astype(f8)
    # global tnT[c*NH+n, p, K, o, j] = tns[c*1024 + n*512 + j, K*256+o*128+p]
    tnT_g = np.ascontiguousarray(
        (tns * F8_SCALE)
        .reshape(NCORES, NH, 512, KT2, 2, 128)
        .transpose(0, 1, 5, 3, 4, 2)
    ).astype(f8).reshape(NCORES * NH, 128, KT2, 2, 512)

    out1 = _get_runner("scores")({"vnT": vnT, "tnT": tnT_g})
    # [NCORES, 128, NH, NM, 8] indices within each 512-wide half
    amax = out1["amax"].reshape(NCORES, 128, NH, NM, 8)[..., :TOPC]
    # vision token id = m*128 + p; entries with p >= M_TILES[m] are padding
    amax = amax.transpose(0, 2, 4, 3, 1).reshape(NCORES, NH, TOPC, NM * 128)
    amax = amax[:, :, :, :LV].astype(np.int64)          # [8, 2, TOPC, 576]
    # exact rescore of every candidate (core, half, rank) text token
    n_global = (
        amax
        + np.arange(NCORES)[:, None, None, None] * LT_SH
        + np.arange(NH)[None, :, None, None] * 512
    ).reshape(NCORES * NH * TOPC, LV)
    cand = np.einsum(
        "cmd,md->cm",
        tns.astype(np.float64)[n_global],
        vn.astype(np.float64),
    ).astype(np.float32)
    scores = cand.max(axis=0)  # [576]

    # ---- host selection (mirrors reference ops; margins >> rescore noise) --
    with jax.default_device(cpu):
        sj = jnp.asarray(scores)
        probs = jax.nn.softmax(sj / TEMP)
        order = jnp.argsort(-probs)
        cum = jnp.cumsum(probs[order])
        thr = int(jnp.sum(cum <= GAMMA))
        sel = np.asarray(order[:thr])

    if thr == 0:
        return np.zeros((0, D), dtype=np.float32)
    uniq = _neighbor_unique(sel)
    S = len(uniq)

    # ---- host: small [S,576] cos-sim + top-k + softmax, bit-exact ----
    with jax.default_device(cpu):
        sel_feat = jnp.asarray(vision_feature[uniq])
        sn = sel_feat / jnp.maximum(
            jnp.linalg.norm(sel_feat, axis=-1, keepdims=True), EPS
        )
        scos = sn @ jnp.asarray(vn).T
        top_vals, top_idx = jax.lax.top_k(scos, TOP_K)
        w = np.asarray(jax.nn.softmax(top_vals, axis=-1))
        top_idx = np.asarray(top_idx)

    W = np.zeros((LV, LV), dtype=np.float32)  # rows: uniq order; cols: vision j
    W[np.arange(S)[:, None], top_idx] = w

    # ---- device program 2: out = W @ vision_feature (bf16), column-sharded --
    bf16 = ml_dtypes.bfloat16
    WT = np.zeros((KV * 128, LV), dtype=np.float32)
    WT[:LV] = W.T
    wT_r = WT.astype(bf16).reshape(KV, 128, LV)  # replicated
    vf_p = np.zeros((KV * 128, D), dtype=np.float32)
    vf_p[:LV] = vision_feature
    # global vf[c*KV+k, p, j] = vf_p[k*128+p, c*512+j]
    vf_g = np.ascontiguousarray(
        vf_p.reshape(KV, 128, NCORES, 512).transpose(2, 0, 1, 3)
    ).astype(bf16).reshape(NCORES * KV, 128, 512)

    out2 = _get_runner("pool")({"wT": wT_r, "vf": vf_g})
    # out is [NCORES*576, 512] bf16: per-core column slices of [576, 4096]
    out_full = (
        out2["out"].astype(np.float32)
        .reshape(NCORES, LV, 512).transpose(1, 0, 2).reshape(LV, D)
    )
    return np.ascontiguousarray(out_full[:S])
